# revision 1
# baseline (speedup 1.0000x reference)
"""Trainium2 Bass kernel for nn_CentralityEncoding (8 NeuronCores, SPMD).

Math (reference):
    out = x + z_in[min(in_deg,511)] + z_out[min(out_deg,511)]
        + sigmoid(cent @ W_cent + b_cent) + sigmoid(vor @ W_vor + b_vor)
        + segment_sum(edge_attr @ W_sum + b_sum, src)

Reformulations:
  * segment_sum(edge_attr @ W_sum + b_sum, src)
        = segment_sum(edge_attr, src) @ W_sum + out_deg_raw[:,None] * b_sum
    (cuts the big matmul from E=1.6M rows to N=50k rows)
  * z_in[deg] = onehot(deg) @ z_in — a K=n_bins matmul accumulated into the
    same PSUM tile as the projection (degrees are tiny vs the 512 table).
  * sigmoid(cent*W + 0) = one ACT op with per-partition scale operand.

Sharding: nodes are split 6250/core; each edge is routed (on host, as part
of sharding) to the core owning its src node, and within a core to the
64-node sub-chunk of that src.  Each sub-chunk's edge list is padded to a
fixed tile capacity so the device program is fully static and identical
across cores — no collectives needed.  Compute is bf16 (f32 PSUM
accumulation); edge features are shipped bf16.

Per-sub-chunk segment sum: for each 128-edge tile build a selection matrix
sel[p,q] = (src_rel[p] == q), q over the 64 sub-chunk nodes, with one DVE
is_equal over a repeating iota, then matmul lhsT=edge_tile [128e,128d],
rhs=sel [128e,64q] accumulating agg[d,q] in PSUM across the sub-chunk's
tiles.  Two sub-chunk aggs concat into [128d,128n]; the projection
lhsT=agg @ W_sum [128d,256] lands node-major in PSUM, z-term matmuls
accumulate on top, and the sigmoid terms + x (DMA-accumulate) complete
the output.
"""

import numpy as np
import ml_dtypes

import bass_rust
import concourse.bass as bass
import concourse.mybir as mybir
import concourse.tile as tile
from concourse.bass_utils import run_bass_kernel_spmd
from concourse.vector_clock import ScopedClock

# ----------------------------------------------------------------------------
# Problem constants (hardcoded per the harness contract).
N_NODES = 50000
N_EDGES = 1600000
NODE_DIM = 256
EDGE_DIM = 128
MAX_DEG = 512  # z tables are [512, 256]; degree clamp is 511
N_CORES = 8
P = 128
NW = 64                        # nodes per sub-chunk (sel compare width)
SUBS = P // NW                 # sub-chunks per 128-node phase-2 group
NPC = N_NODES // N_CORES       # 6250 nodes per core
CH = (NPC + NW - 1) // NW      # sub-chunks per core
NPAIR = CH // SUBS             # 49 groups of 128 nodes for phase 2
NPAD = CH * NW                 # 6272 padded nodes per core
F32 = mybir.dt.float32
BF16 = mybir.dt.bfloat16
I32 = mybir.dt.int32


# ----------------------------------------------------------------------------
# Workarounds for this container's walrus build, which rejects any
# instruction carrying more than ONE semaphore wait ("Too many sync wait
# commands", CoreV3GenImpl setupSyncWait).

# (1) General: split multi-wait instructions during Tile lowering — spill
# all but the last wait onto single-wait NoOps committed just before the
# instruction on the same engine.
_orig_commit = tile.TileContext._commit_instruction


def _commit_split_waits(self, inst, lazy_reg_writes=True):
    si = getattr(inst, "sync_info", None)
    if si is not None and si.on_wait and len(si.on_wait) > 1:
        waits = list(si.on_wait)
        for w in waits[:-1]:
            nop = mybir.InstNoOp(
                name=self.nc.get_next_instruction_name(),
                sync_info=mybir.SyncInfo(on_wait=[w], on_update=[]),
                bass_nofuse=True,
                engine=inst.engine,
            )
            _orig_commit(self, nop, lazy_reg_writes)
        inst.sync_info = mybir.SyncInfo(
            on_wait=[waits[-1]], on_update=list(si.on_update)
        )
    return _orig_commit(self, inst, lazy_reg_writes)


tile.TileContext._commit_instruction = _commit_split_waits


# (2) Kernel-tail Drain: waits are attached after commit; re-emit them
# one-per-NoOp before an empty Drain.
def _patched_drain_and_barrier(self, tick_clock, wait_clock):
    nc = self.nc
    collector = nc.sync.nop(nofuse=True)
    wait_clock.add_sem_waits(
        collector.ins, ScopedClock({None: tick_clock.global_clock})
    )
    si = collector.ins.sync_info
    waits = list(si.on_wait) if si is not None else []
    if waits:
        collector.ins.sync_info = bass_rust.SyncInfo(
            on_wait=[waits[0]], on_update=[]
        )
        for w in waits[1:]:
            nop = nc.sync.nop(nofuse=True)
            nop.ins.sync_info = bass_rust.SyncInfo(on_wait=[w], on_update=[])
    nc.sync.drain()
    nc.all_engine_barrier()
    assert self.sems is not None
    popped = nc._tile_sem_poison_stack.pop()
    assert popped is self._sem_poison
    nc.clear_and_free_semaphores(list(self.sems.allocated().values()))
    nc.all_engine_barrier()


tile.TileContext._drain_and_barrier = _patched_drain_and_barrier


# ----------------------------------------------------------------------------
def build_program(t_cap: int, n_groups: int, has_bsum: bool, has_bcent: bool,
                  has_bvor: bool) -> bass.Bass:
    cap = t_cap * P  # padded edges per sub-chunk
    nc = bass.Bass()

    a_d = nc.declare_dram_parameter("a", [NPAIR * P, SUBS * cap], BF16, isOutput=False)
    srel_d = nc.declare_dram_parameter("srel", [P, CH * t_cap], BF16, isOutput=False)
    x_d = nc.declare_dram_parameter("x", [NPAD, NODE_DIM], F32, isOutput=False)
    idegr_d = nc.declare_dram_parameter("idegrow", [n_groups, NPAD], F32, isOutput=False)
    odegr_d = nc.declare_dram_parameter("odegrow", [n_groups, NPAD], F32, isOutput=False)
    odegraw_d = nc.declare_dram_parameter("odegraw", [P, NPAIR], F32, isOutput=False)
    cent_d = nc.declare_dram_parameter("cent", [P, NPAIR], F32, isOutput=False)
    vor_d = nc.declare_dram_parameter("vor", [P, NPAIR], F32, isOutput=False)
    zin_d = nc.declare_dram_parameter("z_in", [MAX_DEG, NODE_DIM], F32, isOutput=False)
    zout_d = nc.declare_dram_parameter("z_out", [MAX_DEG, NODE_DIM], F32, isOutput=False)
    wsum_d = nc.declare_dram_parameter("W_sum", [EDGE_DIM, NODE_DIM], F32, isOutput=False)
    bsum_d = nc.declare_dram_parameter("b_sum", [1, NODE_DIM], F32, isOutput=False)
    wcent_d = nc.declare_dram_parameter("W_cent", [1, NODE_DIM], F32, isOutput=False)
    bcent_d = nc.declare_dram_parameter("b_cent", [1, NODE_DIM], F32, isOutput=False)
    wvor_d = nc.declare_dram_parameter("W_vor", [1, NODE_DIM], F32, isOutput=False)
    bvor_d = nc.declare_dram_parameter("b_vor", [1, NODE_DIM], F32, isOutput=False)
    out_d = nc.declare_dram_parameter("out", [NPAD, NODE_DIM], F32, isOutput=True)

    sig = mybir.ActivationFunctionType.Sigmoid

    with tile.TileContext(nc) as tc:
        with (
            tc.tile_pool(name="const", bufs=1) as const,
            tc.tile_pool(name="apool", bufs=4) as apool,
            tc.tile_pool(name="spool", bufs=4) as spool,
            tc.tile_pool(name="aggp", bufs=NPAIR) as aggp,
            tc.tile_pool(name="tp", bufs=6) as tp,
            tc.tile_pool(name="op", bufs=7) as op,
            tc.tile_pool(name="psp", bufs=6, space="PSUM") as psp,
            tc.tile_pool(name="prp", bufs=2, space="PSUM") as prp,
        ):
            # --- one-time constants -------------------------------------
            # repeating 0..NW-1 ramp along free dim (bf16 exact for <=255)
            iota2_b = const.tile([P, SUBS * t_cap * NW], BF16, tag="iota2_b")
            nc.gpsimd.iota(iota2_b[:], pattern=[[0, SUBS * t_cap], [1, NW]],
                           base=0, channel_multiplier=0,
                           allow_small_or_imprecise_dtypes=True)
            # partition-index constant over the node axis
            iotap_b = const.tile([P, NPAD], BF16, tag="iotap_b")
            nc.gpsimd.iota(iotap_b[:], pattern=[[0, NPAD]], base=0,
                           channel_multiplier=1,
                           allow_small_or_imprecise_dtypes=True)

            srel_b = const.tile([P, CH * t_cap], BF16, tag="srel_b")
            nc.sync.dma_start(out=srel_b[:], in_=srel_d[:])

            wsum_b = const.tile([EDGE_DIM, NODE_DIM], BF16, tag="wsum_b")
            nc.gpsimd.dma_start(out=wsum_b[:], in_=wsum_d[:])

            # z tables, one [<=128, 256] bf16 slab per degree group
            zin_sb, zout_sb, ideg_oh, odeg_oh = [], [], [], []
            for g in range(n_groups):
                k = min(MAX_DEG - g * P, P)
                zi = const.tile([P, NODE_DIM], BF16, tag=f"zin{g}")
                nc.gpsimd.dma_start(out=zi[:k, :], in_=zin_d[g * P:g * P + k, :])
                zin_sb.append((zi, k))
                zo = const.tile([P, NODE_DIM], BF16, tag=f"zout{g}")
                nc.gpsimd.dma_start(out=zo[:k, :], in_=zout_d[g * P:g * P + k, :])
                zout_sb.append((zo, k))
                # degree rows (value - 128g, offset applied on host),
                # replicated across partitions by the DMA, then one-hot
                # against the partition index
                for name, src, acc in (("i", idegr_d, ideg_oh),
                                       ("o", odegr_d, odeg_oh)):
                    db = const.tile([P, NPAD], BF16, tag=f"degb_{name}{g}")
                    nc.gpsimd.dma_start(
                        out=db[:],
                        in_=src[g:g + 1, :].to_broadcast([P, NPAD]),
                    )
                    oh = const.tile([P, NPAD], BF16, tag=f"oh_{name}{g}")
                    nc.vector.tensor_tensor(out=oh[:], in0=iotap_b[:],
                                            in1=db[:],
                                            op=mybir.AluOpType.is_equal)
                    acc.append(oh)

            def bcast_row(param, tag):
                # sync (HWDGE) keeps these off the gpsimd FIFO
                t = const.tile([P, NODE_DIM], F32, tag=tag)
                nc.sync.dma_start(
                    out=t[:], in_=param[:].to_broadcast([P, NODE_DIM])
                )
                return t

            wc_b = bcast_row(wcent_d, "wc_b")
            wv_b = bcast_row(wvor_d, "wv_b")
            bs_b = bcast_row(bsum_d, "bs_b") if has_bsum else None
            bc_b = bcast_row(bcent_d, "bc_b") if has_bcent else None
            bv_b = bcast_row(bvor_d, "bv_b") if has_bvor else None

            cent_sb = const.tile([P, NPAIR], F32, tag="cent_sb")
            nc.sync.dma_start(out=cent_sb[:], in_=cent_d[:])
            vor_sb = const.tile([P, NPAIR], F32, tag="vor_sb")
            nc.sync.dma_start(out=vor_sb[:], in_=vor_d[:])
            if has_bsum:
                odegraw_sb = const.tile([P, NPAIR], F32, tag="odegraw_sb")
                nc.sync.dma_start(out=odegraw_sb[:], in_=odegraw_d[:])

            # --- main pipeline: pairs of 64-node sub-chunks ---------------
            # Phase 2 for pair cp is emitted LAG pairs after its phase 1 so
            # DVE's sel builds never queue behind adds that wait on PE/ACT.
            agg_tiles: dict[int, object] = {}
            ps_tiles: dict[int, list] = {}

            def phase1(cp):
                # one DMA for the group's SUBS sub-chunks (contiguous rows)
                at = apool.tile([P, SUBS * cap], BF16)
                nc.sync.dma_start(
                    out=at[:], in_=a_d[cp * P:(cp + 1) * P, :]
                )
                # one is_equal for all SUBS sub-chunks
                st = spool.tile([P, SUBS * t_cap * NW], BF16)
                srel_3d = srel_b[:, cp * SUBS * t_cap:(cp + 1) * SUBS * t_cap][:, :, None]
                nc.vector.tensor_tensor(
                    out=st[:].rearrange("p (t d) -> p t d", t=SUBS * t_cap),
                    in0=srel_3d.to_broadcast([P, SUBS * t_cap, NW]),
                    in1=iota2_b[:].rearrange("p (t d) -> p t d", t=SUBS * t_cap),
                    op=mybir.AluOpType.is_equal,
                )
                ps_tiles[cp] = []
                for s in range(SUBS):
                    ps = psp.tile([P, NW], F32, space="PSUM")
                    ps_tiles[cp].append(ps)
                    for t in range(t_cap):
                        tt = s * t_cap + t
                        nc.tensor.matmul(
                            out=ps[:],
                            lhsT=at[:, tt * P:(tt + 1) * P],
                            rhs=st[:, tt * NW:(tt + 1) * NW],
                            start=(t == 0),
                            stop=(t == t_cap - 1),
                        )

            def phase1_cast(cp):
                # lagged so DVE's sel builds don't queue behind PSUM waits
                aggt = aggp.tile([P, P], BF16)
                agg_tiles[cp] = aggt
                for s, ps in enumerate(ps_tiles.pop(cp)):
                    nc.vector.tensor_copy(aggt[:, s * NW:(s + 1) * NW], ps[:])

            def phase2(cp):
                aggt = agg_tiles.pop(cp)
                # node-major accumulation in PSUM:
                #   agg^T @ W_sum + onehot_in^T @ z_in + onehot_out^T @ z_out
                pp = prp.tile([P, NODE_DIM], F32, space="PSUM")
                nsl = slice(cp * P, (cp + 1) * P)
                nc.tensor.matmul(out=pp[:], lhsT=aggt[:], rhs=wsum_b[:],
                                 start=True, stop=False, skip_group_check=True)
                for g in range(n_groups):
                    zi, k = zin_sb[g]
                    nc.tensor.matmul(out=pp[:], lhsT=ideg_oh[g][:k, nsl],
                                     rhs=zi[:k, :], start=False, stop=False,
                                     skip_group_check=True)
                    zo, k = zout_sb[g]
                    last = g == n_groups - 1
                    nc.tensor.matmul(out=pp[:], lhsT=odeg_oh[g][:k, nsl],
                                     rhs=zo[:k, :], start=False, stop=last,
                                     skip_group_check=True)

                # sigmoid terms: one ACT op each (bias-free fast path)
                ct = tp.tile([P, NODE_DIM], F32, tag="ct")
                if has_bcent:
                    nc.vector.tensor_mul(
                        ct[:], wc_b[:],
                        cent_sb[:, cp:cp + 1].to_broadcast([P, NODE_DIM]))
                    nc.vector.tensor_add(ct[:], ct[:], bc_b[:])
                    nc.scalar.activation(out=ct[:], in_=ct[:], func=sig)
                else:
                    nc.scalar.activation(out=ct[:], in_=wc_b[:], func=sig,
                                         scale=cent_sb[:, cp:cp + 1])
                vt = tp.tile([P, NODE_DIM], F32, tag="vt")
                if has_bvor:
                    nc.vector.tensor_mul(
                        vt[:], wv_b[:],
                        vor_sb[:, cp:cp + 1].to_broadcast([P, NODE_DIM]))
                    nc.vector.tensor_add(vt[:], vt[:], bv_b[:])
                    nc.scalar.activation(out=vt[:], in_=vt[:], func=sig)
                else:
                    nc.scalar.activation(out=vt[:], in_=wv_b[:], func=sig,
                                         scale=vor_sb[:, cp:cp + 1])

                o = op.tile([P, NODE_DIM], F32)
                # gpsimd on purpose: DVE is the bottleneck engine and both
                # operands are SBUF (Pool cannot read PSUM)
                nc.gpsimd.tensor_add(o[:], ct[:], vt[:])
                if has_bsum:
                    bst = tp.tile([P, NODE_DIM], F32, tag="bst")
                    nc.vector.tensor_mul(
                        bst[:], bs_b[:],
                        odegraw_sb[:, cp:cp + 1].to_broadcast([P, NODE_DIM]))
                    nc.vector.tensor_add(o[:], o[:], bst[:])
                nc.vector.tensor_add(o[:], o[:], pp[:])
                # x lands via DMA-accumulate (SWDGE), saving a DVE add
                nc.gpsimd.dma_start(out=o[:], in_=x_d[nsl, :],
                                    accum_op=mybir.AluOpType.add)
                o_tiles[cp] = o

            def phase3(cp):
                # out-store stays on sync (pure-load queue) but is emitted
                # lagged so its wait is satisfied at issue time and never
                # stalls later a-loads in the HWDGE FIFO
                o = o_tiles.pop(cp)
                nc.sync.dma_start(out=out_d[cp * P:(cp + 1) * P, :], in_=o[:])

            o_tiles: dict[int, object] = {}
            CAST_LAG, LAG, SLAG = 2, 6, 10
            for cp in range(NPAIR + SLAG):
                if cp < NPAIR:
                    phase1(cp)
                if CAST_LAG <= cp < NPAIR + CAST_LAG:
                    phase1_cast(cp - CAST_LAG)
                if LAG <= cp < NPAIR + LAG:
                    phase2(cp - LAG)
                if cp >= SLAG:
                    phase3(cp - SLAG)

    return nc


# ----------------------------------------------------------------------------
def prepare_inputs(x, edge_index, edge_attr, voronoi_values, centralities,
                   z_in, z_out, W_cent, b_cent, W_vor, b_vor, W_sum, b_sum):
    """Host-side sharding: bucket edges by owning (core, 64-node sub-chunk)
    of their src node, pad each sub-chunk to a fixed tile capacity.
    Returns (in_maps, build_key)."""
    src = np.asarray(edge_index[0], dtype=np.int64)
    dst = np.asarray(edge_index[1], dtype=np.int64)
    edge_attr = np.asarray(edge_attr, dtype=np.float32)
    x = np.asarray(x, dtype=np.float32)

    out_deg_raw = np.bincount(src, minlength=N_NODES).astype(np.int64)
    in_deg_raw = np.bincount(dst, minlength=N_NODES).astype(np.int64)
    in_deg = np.minimum(in_deg_raw, MAX_DEG - 1).astype(np.int32)
    out_deg = np.minimum(out_deg_raw, MAX_DEG - 1).astype(np.int32)
    n_groups = max(int(max(in_deg.max(), out_deg.max())) // P + 1, 1)

    core_of = src // NPC
    within = src - core_of * NPC
    gchunk = core_of * CH + within // NW     # [E] in [0, N_CORES*CH)
    srel = (within % NW).astype(np.float32)  # [E] in [0,NW)

    order = np.argsort(gchunk, kind="stable")
    gchunk_s = gchunk[order]
    n_chunks = N_CORES * CH
    counts = np.bincount(gchunk_s, minlength=n_chunks)
    t_cap = max(int(np.ceil(counts.max() / P)), 1)
    cap = t_cap * P

    starts = np.zeros(n_chunks, dtype=np.int64)
    starts[1:] = np.cumsum(counts)[:-1]
    slot = np.arange(N_EDGES, dtype=np.int64) - starts[gchunk_s]
    # a: flat row index in the [n_groups, P(partition), SUBS, t_cap] grid so
    # a group's sub-chunks are contiguous per partition row
    ggrp = gchunk_s // SUBS
    sub = gchunk_s % SUBS
    pos_a = (ggrp * P + slot % P) * (SUBS * t_cap) + sub * t_cap + slot // P
    # srel: flat index in the [n_chunks, P(partition), t_cap] grid
    pos_s = (gchunk_s * P + slot % P) * t_cap + slot // P

    a_grid = np.zeros((n_chunks * P * t_cap, EDGE_DIM), dtype=ml_dtypes.bfloat16)
    a_grid[pos_a] = edge_attr[order].astype(ml_dtypes.bfloat16)
    srel_grid = np.full(n_chunks * P * t_cap, -1.0, dtype=ml_dtypes.bfloat16)
    srel_grid[pos_s] = srel[order].astype(ml_dtypes.bfloat16)
    a_grid = a_grid.reshape(N_CORES, NPAIR * P, SUBS * cap)
    srel_grid = srel_grid.reshape(N_CORES, CH, P, t_cap)

    def pad_nodes(arr, fill=0):
        # [N_NODES, ...] -> [N_CORES, NPAD, ...]
        shp = (N_CORES, NPAD) + arr.shape[1:]
        outa = np.full(shp, fill, dtype=arr.dtype)
        outa[:, :NPC] = arr.reshape((N_CORES, NPC) + arr.shape[1:])
        return outa

    x_p = pad_nodes(x)
    # per-group degree rows: value - 128*g (no match against iota 0..127
    # outside the group's range)
    ideg_p = pad_nodes(in_deg.astype(np.float32)[:, None])[..., 0]
    odeg_p = pad_nodes(out_deg.astype(np.float32)[:, None])[..., 0]
    goff = (np.arange(n_groups, dtype=np.float32) * P)[None, :, None]
    ideg_rows = np.ascontiguousarray(ideg_p[:, None, :] - goff)
    odeg_rows = np.ascontiguousarray(odeg_p[:, None, :] - goff)
    odegraw_p = pad_nodes(out_deg_raw.astype(np.float32)[:, None])
    cent_p = pad_nodes(np.asarray(centralities, dtype=np.float32))
    vor_p = pad_nodes(np.asarray(voronoi_values, dtype=np.float32))

    def col_layout(a):  # [NPAD,1] -> [P, NPAIR]  (partition-major)
        return np.ascontiguousarray(a.reshape(NPAIR, P).T)

    z_in = np.asarray(z_in, dtype=np.float32)
    z_out = np.asarray(z_out, dtype=np.float32)
    W_sum = np.asarray(W_sum, dtype=np.float32)
    row = lambda v: np.ascontiguousarray(
        np.asarray(v, dtype=np.float32).reshape(1, NODE_DIM))
    b_sum_r, b_cent_r, b_vor_r = row(b_sum), row(b_cent), row(b_vor)
    flags = (bool(np.any(b_sum_r)), bool(np.any(b_cent_r)),
             bool(np.any(b_vor_r)))

    in_maps = []
    for c in range(N_CORES):
        in_maps.append({
            "a": np.ascontiguousarray(a_grid[c]),
            "srel": np.ascontiguousarray(
                srel_grid[c].transpose(1, 0, 2).reshape(P, CH * t_cap)),
            "x": np.ascontiguousarray(x_p[c]),
            "idegrow": ideg_rows[c],
            "odegrow": odeg_rows[c],
            "odegraw": col_layout(odegraw_p[c]),
            "cent": col_layout(cent_p[c]),
            "vor": col_layout(vor_p[c]),
            "z_in": z_in,
            "z_out": z_out,
            "W_sum": W_sum,
            "b_sum": b_sum_r,
            "W_cent": row(W_cent),
            "b_cent": b_cent_r,
            "W_vor": row(W_vor),
            "b_vor": b_vor_r,
        })
    return in_maps, (t_cap, n_groups) + flags


_PROGRAM_CACHE: dict[tuple, bass.Bass] = {}


def kernel(**inputs) -> np.ndarray:
    in_maps, key = prepare_inputs(**inputs)
    nc = _PROGRAM_CACHE.get(key)
    if nc is None:
        nc = build_program(*key)
        _PROGRAM_CACHE[key] = nc
    res = None
    for attempt in range(3):
        try:
            res = run_bass_kernel_spmd(nc, in_maps, core_ids=list(range(N_CORES)))
            break
        except Exception:
            # axon transiently reports "accelerator device unrecoverable";
            # a clean retry succeeds
            if attempt == 2:
                raise
    out = np.concatenate(
        [res.results[i]["out"][:NPC] for i in range(N_CORES)], axis=0
    )
    return out.astype(np.float32)

